# revision 31
# baseline (speedup 1.0000x reference)
"""MI-loss kernel for Trainium2 (8 NeuronCores, SPMD data-parallel).

Math (matches the jax reference):
  probs = softmax(router_logits, axis=-1)            # [B, S, E]
  All S tokens of batch b share label L[b], so
    seg[t]    = sum_{b: L[b]=t} bsum[b],  bsum[b] = sum_s probs[b, s]   # [E]
    counts[t] = S * |{b: L[b]=t}|
  followed by a tiny [T, E] mutual-information reduction to a scalar.

Device work per core: per-batch sums of softmax probs over 4 batches
(2M elements).  The MI loss averages ~2048 tokens per (task, expert)
cell, so a bf16 input cast is numerically free (measured end-to-end
rel-err ~4e-7) and halves the HBM stream to 4 MiB/core.

Host prep: cast to bf16 and pack partition-major so each partition's
data is one contiguous HBM run: dev_x[p, g, e] with g = b*64 + t the
global token-segment (partition p of batch b holds tokens p*64..p*64+63).
Every input DMA is then 128 descriptors of (nseg*128) contiguous bytes.

Device pipeline (ACT-bound; exp floor is 13.7us at 1 elem/cycle/lane):
  - 6 tapered input DMAs on the sync HWDGE ring, issued upfront.  Front
    chunks are big: SDMA engine 15 stalls sporadically for the first
    ~6us of any stream (known trn2 quirk) and every chunk's completion
    sem waits on all 16 engines, so small early chunks each pay ~2.5us
    of straggler latency.
  - ACT: exp chunk -> pt bf16 (one big [128, 256, 64] tile; sub-range
    writes so downstream ops get range-level deps).  Exp table preloaded
    via a dummy activation before data arrives.  ACT does nothing else
    until its exps finish.
  - DVE: double fold (64->32->16 via 2x-rate bf16 tensor_adds) then a
    1x-rate 16-wide reduce; reciprocal via the approx-fast custom op +
    bf16 cast for big chunks (exact RECIPROCAL is 8 cyc/elem), exact
    single-op reciprocal for small ones (fewer-instruction crossover).
    DVE is the back-half critical path: nothing else is allowed on it.
  - PE : blocked normalization-fold, 8 segs per matmul (moving-dim cap
    512): ps[8, 512] += rb_blk[128, 8].T @ pt_blk[128, 512], accumulated
    across a batch's 8 blocks in one PSUM bank; only diagonal [1, 64]
    blocks are wanted (discarded on host).  A short junk-matmul warmup
    lifts PE's cold p-state; the dense tail bursts ramp to the full
    2.4 GHz on their own (the ramp resets on any idle gap).
  - PSUM -> SBUF copies all on ACT (GPSIMD cannot read PSUM on trn2;
    ACT is free after its exps and the copies overlap DVE's drain);
    per-batch output DMAs on the sync ring behind the input DMAs.
The label-dependent segment-sum + tiny MI formula run on host: all 8192
tokens of a batch share one label, so only [32, 64] per-batch sums are
needed from the device.
"""

import numpy as np

_B, _S, _E = 32, 8192, 64
_NT = 8  # num tasks
_TOPK = 2.0
_WMI = 0.01
_EPS = 1e-4
_NCORES = 8
_BPC = _B // _NCORES  # batches per core
_P = 128
_TPB = _S // _P  # token segments per batch (64)
_SEGS = _BPC * _TPB  # global segments per core (256)
_M = 8  # segs folded per matmul block (moving-dim cap 512 = 8*64)
_W = _M * _E

# chunk boundaries (global segs): DMA + exp granularity.  SDMA engine 15
# stalls sporadically for the first ~6us of the stream (known trn2 quirk)
# and every chunk's completion sem waits for all 16 engines, so each early
# chunk pays ~2.5us of straggler latency: front chunks are big to amortize
# it.  Sizes taper down through the back half so DVE's fold backlog drains
# before the serial tail.
_CB = [0, 32, 96, 160, 208, 240, 256]
_CHUNKS = list(zip(_CB[:-1], _CB[1:]))

_nc_cache = {}


def _build_nc():
    import concourse.tile as tile
    from concourse import bacc, mybir

    f32 = mybir.dt.float32
    bf16 = mybir.dt.bfloat16

    nc = bacc.Bacc("TRN2", target_bir_lowering=False, debug=False)
    x = nc.dram_tensor("x", [_P, _SEGS, _E], bf16, kind="ExternalInput")
    out = nc.dram_tensor("out", [_M, _BPC * _W], f32, kind="ExternalOutput")

    with tile.TileContext(nc) as tc:
        with (
            tc.tile_pool(name="xin", bufs=1) as xpool,
            tc.tile_pool(name="big", bufs=1) as big,
            tc.tile_pool(name="acc", bufs=1, space="PSUM") as psum_pool,
        ):
            pt = big.tile([_P, _SEGS, _E], bf16, tag="pt")
            f1 = big.tile([_P, _SEGS, _E // 2], bf16, tag="f1")
            f2 = big.tile([_P, _SEGS, _E // 4], bf16, tag="f2")
            st = big.tile([_P, _SEGS], f32, tag="st")
            rf = big.tile([_P, _SEGS], f32, tag="rf")
            rb = big.tile([_P, _SEGS], bf16, tag="rb")
            out_sb = big.tile([_M, _BPC * _W], f32, tag="osb")
            # PE p-state warmup scratch (memset early on idle engines)
            wa = big.tile([_P, _M], bf16, tag="wa")
            wb = big.tile([_P, _W], bf16, tag="wb")
            warm = big.tile([1, 1], f32, tag="warm")

            # exp spline table preload: first ACTIVATE triggers the
            # ~1.3us table load; run it on a 1-element tile before data
            nc.vector.memset(warm[:], 0.0)
            nc.scalar.activation(
                out=warm[:], in_=warm[:], func=mybir.ActivationFunctionType.Exp
            )
            nc.gpsimd.memset(wa[:], 0.0)
            nc.gpsimd.memset(wb[:], 0.0)

            # all input loads upfront on the sync HWDGE ring
            xts = []
            for ci, (s0, s1) in enumerate(_CHUNKS):
                xt = xpool.tile(
                    [_P, s1 - s0, _E], bf16, tag=f"xt{ci}", name=f"xt{ci}"
                )
                nc.sync.dma_start(out=xt[:], in_=x[:, s0:s1, :])
                xts.append(xt)

            # PE p-state warmup: ~7 junk matmuls fill the otherwise-idle
            # pre-stream window so real matmuls start at full clock
            # a short warm burst lifts PE out of its cold p-state before the
            # first real matmul (the ramp resets on any idle gap, so a long
            # warmup can't hold full clock — the tail bursts ramp on their
            # own once matmuls pack back-to-back)
            wps = psum_pool.tile([_M, _W], f32, tag="wps")
            for _ in range(4):
                nc.tensor.matmul(wps[:], wa[:], wb[:], start=True, stop=True)

            ps = [
                psum_pool.tile([_M, _W], f32, name=f"ps{b}", tag=f"ps{b}")
                for b in range(_BPC)
            ]
            for ci, (s0, s1) in enumerate(_CHUNKS):
                nc.scalar.activation(
                    out=pt[:, s0:s1, :],
                    in_=xts[ci][:],
                    func=mybir.ActivationFunctionType.Exp,
                )
                # bf16 denominators: per-token rounding is independent
                # across 8192 tokens and averages out in the batch sums.
                # Double fold at the TT 2x bf16 rate then a 16-wide 1x
                # reduce: 40 cycles/token vs 48 single-fold, and fewer
                # instructions than deeper fold chains
                with nc.allow_low_precision("bf16 softmax denominators"):
                    nc.vector.tensor_add(
                        f1[:, s0:s1, :],
                        pt[:, s0:s1, 0 : _E // 2],
                        pt[:, s0:s1, _E // 2 : _E],
                    )
                    nc.vector.tensor_add(
                        f2[:, s0:s1, :],
                        f1[:, s0:s1, 0 : _E // 4],
                        f1[:, s0:s1, _E // 4 : _E // 2],
                    )
                    nc.vector.reduce_sum(
                        out=st[:, s0:s1],
                        in_=f2[:, s0:s1, :],
                        axis=mybir.AxisListType.X,
                    )
                    # approx recip is ~5x cheaper per element than exact
                    # RECIPROCAL (8 cyc/elem HW iterative divide) but needs
                    # a separate fp32->bf16 cast op; the crossover vs the
                    # one-instruction exact path is around 32 segs.  51-ULP
                    # fp32 is far tighter than the surrounding bf16; dens
                    # are in [1, 4e3], clear of denorm/inf edge cases
                    if s1 - s0 >= 32:
                        nc.vector.reciprocal_approx_fast(
                            out=rf[:, s0:s1], in_=st[:, s0:s1]
                        )
                        nc.vector.tensor_copy(out=rb[:, s0:s1], in_=rf[:, s0:s1])
                    else:
                        nc.vector.reciprocal(out=rb[:, s0:s1], in_=st[:, s0:s1])
                # normalization-fold matmuls: full m-blocks inside chunk,
                # sub-m tail accumulates into the psum corner
                g = s0
                while g < s1:
                    b = g // _TPB
                    mb = min(_M, s1 - g, (b + 1) * _TPB - g)
                    nc.tensor.matmul(
                        ps[b][0:mb, 0 : mb * _E],
                        rb[:, g : g + mb],
                        pt[:, g : g + mb, :],
                        start=(g == b * _TPB),
                        stop=(g + mb == (b + 1) * _TPB),
                    )
                    g += mb
                    if g % _TPB == 0:
                        bb = g // _TPB - 1
                        # batch closed: copy psum to sbuf on ACT — GPSIMD
                        # can't read PSUM on trn2, and DVE's fold queue is
                        # the back-half critical path; ACT is free after
                        # its exps and the copies overlap DVE's drain
                        nc.scalar.copy(
                            out=out_sb[:, bb * _W : (bb + 1) * _W],
                            in_=ps[bb][:],
                        )
                        nc.sync.dma_start(
                            out=out[:, bb * _W : (bb + 1) * _W],
                            in_=out_sb[:, bb * _W : (bb + 1) * _W],
                        )
    nc.compile()
    return nc


def _get_nc():
    if "nc" not in _nc_cache:
        _nc_cache["nc"] = _build_nc()
    return _nc_cache["nc"]


def _extract_bsum(arr):
    """arr [m, bpc*m*64] -> [bpc, 64]: sum the diagonal [1, 64] blocks."""
    out = np.empty((_BPC, _E), np.float32)
    idx = np.arange(_M)
    for b in range(_BPC):
        blk = arr[:, b * _W : (b + 1) * _W].reshape(_M, _M, _E)
        out[b] = blk[idx, idx, :].sum(axis=0, dtype=np.float32)
    return out


def _prep_inputs(logits_np):
    """[B, S, E] f32 -> per-core [128, 256, 64] bf16, partition-major."""
    import ml_dtypes

    xr = logits_np.reshape(_NCORES, _BPC, _P, _TPB, _E)
    xr = np.ascontiguousarray(xr.transpose(0, 2, 1, 3, 4))
    return xr.reshape(_NCORES, _P, _SEGS, _E).astype(ml_dtypes.bfloat16)


def _run_device(logits_np, trace=False):
    """logits_np [B, S, E] f32 -> bsum [B, E] f32 (per-batch softmax sums)."""
    from concourse.bass_utils import run_bass_kernel_spmd

    nc = _get_nc()
    xb = _prep_inputs(logits_np)
    in_maps = [{"x": xb[c]} for c in range(_NCORES)]
    res = run_bass_kernel_spmd(nc, in_maps, list(range(_NCORES)), trace=trace)
    bsum = np.concatenate(
        [_extract_bsum(res.results[c]["out"]) for c in range(_NCORES)], axis=0
    )
    return bsum, res


def _mi_from_bsum(bsum, labels):
    bsum = bsum.astype(np.float32)
    seg = np.zeros((_NT, _E), np.float32)
    np.add.at(seg, labels, bsum)
    counts = (np.bincount(labels, minlength=_NT) * float(_S)).astype(np.float32)
    mi_gate = seg * counts[:, None]
    tot = mi_gate.sum(dtype=np.float32) / np.float32(_TOPK)
    mi_gate = mi_gate / (tot + np.float32(_EPS))
    p_ti = mi_gate.sum(axis=1, keepdims=True, dtype=np.float32) + np.float32(_EPS)
    p_ei = mi_gate.sum(axis=0, keepdims=True, dtype=np.float32) + np.float32(_EPS)
    mi_loss = -(
        mi_gate * np.log(mi_gate / p_ti / p_ei + np.float32(_EPS))
    ).sum(dtype=np.float32)
    return np.asarray(np.float32(_WMI) * mi_loss, dtype=np.float32)


def kernel(router_logits, router_labels):
    import time

    logits = np.asarray(router_logits, dtype=np.float32)
    labels = np.asarray(router_labels).astype(np.int64)
    last_err = None
    for attempt in range(3):
        try:
            bsum, _ = _run_device(logits)
            return _mi_from_bsum(bsum, labels)
        except Exception as e:  # transient NRT device errors observed
            last_err = e
            time.sleep(2.0 * (attempt + 1))
    raise last_err
